# revision 4
# baseline (speedup 1.0000x reference)
"""Trainium2 Bass kernel for the vq_codebook problem (nn_CGCLR_72370198937695).

Math (per row b of the batch):
    aug    = [x_b, 1]                                   # [513]
    h1     = relu(x_b @ W1 + b1)                        # [2048]
    h2     = relu(h1 @ W2 + b2)                         # [2048]
    w_hat  = h2 @ W3 + b3                               # [513]
    proj   = aug . w_hat                                # scalar
    s_j    = aug . codebook_j                           # [1024]
    idx    = argmin_j (proj - s_j)^2  == argmax_j -(s_j - proj)^2
    w_tilde= codebook[idx]
    y_tilde= aug . w_tilde

Distribution: data-parallel over 8 NeuronCores, batch 16384 -> 2048 rows/core.
Weights + codebook replicated; no collectives. All matmuls fp32 (full
precision; fp32 runs at 4 cycles/row on the PE -> the kernel is PE-bound).

Per-core schedule, batch tiles of 512 (the matmul moving free dim):
    L1/L2 keep activations TRANSPOSED (feature-major):
        h1T[m-chunk] [128, 512] = (W1[k,m] chunk stationary).T @ xT-moving
    L3 + codebook scores are batch-major:
        w_hat rows = (h2T chunk stationary).T @ W3-moving
        scores rows = (xT chunk stationary).T @ cbT-moving
    argmin via DVE max/max_index on u = -(s - proj)^2
    w_tilde gathered from DRAM codebook by indirect DMA.
"""
import sys

for _p in ("/opt/trn_rl_repo", "/root/.axon_site"):
    if _p not in sys.path:
        sys.path.insert(0, _p)

import numpy as np
import concourse.bacc as bacc
import concourse.bass as bass
import concourse.tile as tile
from concourse import mybir
from concourse.bass_utils import run_bass_kernel_spmd

F32 = mybir.dt.float32
I32 = mybir.dt.int32
U32 = mybir.dt.uint32
AF = mybir.ActivationFunctionType
OP = mybir.AluOpType

NCORES = 8
B = 16384
D = 512       # input dim
H = 2048      # hidden dim
E = 1024      # codebook entries
A = 513       # aug dim = D + 1
P = 128       # partitions
BSH = B // NCORES   # rows per core = 2048
BT = 512            # batch tile (matmul moving free dim)
NT = BSH // BT      # 4 batch tiles per core
KD = D // P         # 4 contraction chunks for input dim
KH = H // P         # 16 contraction chunks for hidden dim
MH = H // P         # 16 output chunks per hidden layer
RG = BT // P        # 4 row-groups of 128 per batch tile


def _bcast(handle, offset, n_free):
    """DRAM AP broadcasting one row across all 128 partitions."""
    base = handle[:]
    return bass.AP(tensor=base.tensor, offset=offset, ap=[[0, P], [1, n_free]])


def build_program():
    nc = bacc.Bacc("TRN2", target_bir_lowering=False)

    x_d = nc.dram_tensor("x", [BSH, D], F32, kind="ExternalInput")
    xT_d = nc.dram_tensor("xT", [D, BSH], F32, kind="ExternalInput")
    W1_d = nc.dram_tensor("W1", [D, H], F32, kind="ExternalInput")
    W2_d = nc.dram_tensor("W2", [H, H], F32, kind="ExternalInput")
    W3_d = nc.dram_tensor("W3", [H, A], F32, kind="ExternalInput")
    b1_d = nc.dram_tensor("b1", [H], F32, kind="ExternalInput")
    b2_d = nc.dram_tensor("b2", [H], F32, kind="ExternalInput")
    b3_d = nc.dram_tensor("b3", [A], F32, kind="ExternalInput")
    cbT_d = nc.dram_tensor("cbT", [A, E], F32, kind="ExternalInput")
    cb_d = nc.dram_tensor("cb", [E, A], F32, kind="ExternalInput")

    o_wh = nc.dram_tensor("w_hat", [BSH, A], F32, kind="ExternalOutput")
    o_wt = nc.dram_tensor("w_tilde", [BSH, A], F32, kind="ExternalOutput")
    o_ix = nc.dram_tensor("cluster", [BSH, 1], I32, kind="ExternalOutput")
    o_y = nc.dram_tensor("y_tilde", [BSH, 1], F32, kind="ExternalOutput")

    with tile.TileContext(nc) as tc:
        with (
            tc.tile_pool(name="const", bufs=1) as const,
            tc.tile_pool(name="xt", bufs=2) as xt_p,
            tc.tile_pool(name="h1", bufs=1) as h1_p,
            tc.tile_pool(name="h2", bufs=1) as h2_p,
            tc.tile_pool(name="wp", bufs=3) as wp_p,
            tc.tile_pool(name="wh", bufs=6) as wh_p,
            tc.tile_pool(name="sc", bufs=2) as sc_p,
            tc.tile_pool(name="wt", bufs=2) as wt_p,
            tc.tile_pool(name="xr", bufs=2) as xr_p,
            tc.tile_pool(name="sm", bufs=3) as sm_p,
            tc.tile_pool(name="ps", bufs=8, space="PSUM") as ps_p,
        ):
            # ---------------- constants ----------------
            cbT_sb = const.tile([P, KD, E], F32)
            for k in range(KD):
                nc.sync.dma_start(cbT_sb[:, k, :], cbT_d[k * P:(k + 1) * P, :])
            c_rep = const.tile([P, E], F32)          # codebook[:,512] replicated
            nc.sync.dma_start(c_rep[:], _bcast(cbT_d, D * E, E))
            b3_rep = const.tile([P, A], F32)
            nc.sync.dma_start(b3_rep[:], _bcast(b3_d, 0, A))
            b1_sb = const.tile([P, KH], F32)         # b1_sb[p,m] = b1[m*128+p]
            nc.sync.dma_start(
                b1_sb[:],
                bass.AP(tensor=b1_d[:].tensor, offset=0, ap=[[1, P], [P, KH]]),
            )
            b2_sb = const.tile([P, KH], F32)
            nc.sync.dma_start(
                b2_sb[:],
                bass.AP(tensor=b2_d[:].tensor, offset=0, ap=[[1, P], [P, KH]]),
            )
            w3l_sb = const.tile([P, KH], F32)    # w3l[p,k] = W3[k*128+p, 512]
            nc.sync.dma_start(
                w3l_sb[:],
                bass.AP(tensor=W3_d[:].tensor, offset=D, ap=[[A, P], [A * P, KH]]),
            )

            # ---------------- per-tile pipeline ----------------
            def mlp_layer(KC, rhs_tile, W_dram, bias_sb, out_pool, out_tag):
                """out (feature-major [128, MH, BT] f32) = relu(W-chunk.T @ rhs + b)."""
                out = out_pool.tile([P, MH, BT], F32, tag=out_tag)
                for mg in range(2):  # groups of 8 m-chunks -> 8 live psum banks
                    pss = [ps_p.tile([P, BT], F32, tag="mm", name=f"ps_l{KC}_{mg}_{i}") for i in range(8)]
                    for k in range(KC):
                        wp = wp_p.tile([P, 1024], F32, tag="wp")
                        nc.sync.dma_start(
                            wp[:],
                            W_dram[k * P:(k + 1) * P, mg * 1024:(mg + 1) * 1024],
                        )
                        for m in range(8):
                            nc.tensor.matmul(
                                pss[m][:],
                                wp[:, m * P:(m + 1) * P],
                                rhs_tile[:, k, :],
                                start=(k == 0),
                                stop=(k == KC - 1),
                            )
                    for m in range(8):
                        mi = mg * 8 + m
                        nc.scalar.activation(
                            out[:, mi, :],
                            pss[m][:],
                            AF.Relu,
                            bias=bias_sb[:, mi:mi + 1],
                        )
                return out

            def l3(h2t):
                """w_hat batch-major per rg: [128, 513] tiles."""
                whs = [wh_p.tile([P, A], F32, tag="wh", name=f"wh_{i}") for i in range(RG)]
                pss = [ps_p.tile([P, BT], F32, tag="mm", name=f"ps_l3_{i}") for i in range(RG)]
                for k in range(KH):
                    wp = wp_p.tile([P, A], F32, tag="wp")
                    nc.sync.dma_start(wp[:], W3_d[k * P:(k + 1) * P, :])
                    for rg in range(RG):
                        nc.tensor.matmul(
                            pss[rg][:],
                            h2t[:, k, rg * P:(rg + 1) * P],
                            wp[:, 0:512],
                            start=(k == 0),
                            stop=(k == KH - 1),
                        )
                for rg in range(RG):
                    nc.vector.tensor_tensor(
                        whs[rg][:, 0:512], pss[rg][:], b3_rep[:, 0:512], op=OP.add
                    )
                # tail column 512: sequential accumulation group per rg
                for rg in range(RG):
                    pst = ps_p.tile([P, 2], F32, tag="mm", name=f"ps_tail_{rg}")
                    for k in range(KH):
                        nc.tensor.matmul(
                            pst[:, 0:1],
                            h2t[:, k, rg * P:(rg + 1) * P],
                            w3l_sb[:, k:k + 1],
                            start=(k == 0),
                            stop=(k == KH - 1),
                        )
                    nc.vector.tensor_tensor(
                        whs[rg][:, 512:513], pst[:, 0:1], b3_rep[:, 512:513],
                        op=OP.add,
                    )
                return whs

            def scores_epilogue(xt, whs, t):
                for rg in range(RG):
                    row0 = t * BT + rg * P
                    wh = whs[rg]
                    sc = sc_p.tile([P, E], F32, tag="sc")
                    for nb in range(2):
                        ps = ps_p.tile([P, BT], F32, tag="mm")
                        for k in range(KD):
                            nc.tensor.matmul(
                                ps[:],
                                xt[:, k, rg * P:(rg + 1) * P],
                                cbT_sb[:, k, nb * 512:(nb + 1) * 512],
                                start=(k == 0),
                                stop=(k == KD - 1),
                            )
                        nc.vector.tensor_tensor(
                            sc[:, nb * 512:(nb + 1) * 512],
                            ps[:],
                            c_rep[:, nb * 512:(nb + 1) * 512],
                            op=OP.add,
                        )
                    xr = xr_p.tile([P, D], F32, tag="xr")
                    nc.sync.dma_start(xr[:], x_d[row0:row0 + P, :])
                    # proj = sum(x*w_hat[:512]) + w_hat[512]
                    scr = sm_p.tile([P, D], F32, tag="scr")
                    projp = sm_p.tile([P, 1], F32, tag="projp")
                    nc.vector.scalar_tensor_tensor(
                        out=scr[:], in0=wh[:, 0:512], scalar=1.0, in1=xr[:],
                        op0=OP.mult, op1=OP.mult, accum_out=projp[:],
                    )
                    negp = sm_p.tile([P, 1], F32, tag="negp")
                    nc.vector.tensor_scalar(
                        negp[:], projp[:], wh[:, 512:513], -1.0, OP.add, OP.mult
                    )
                    # u = -(s - proj)^2 ; argmax u == argmin distance
                    a_t = sc_p.tile([P, E], F32, tag="a")
                    nc.gpsimd.tensor_scalar(a_t[:], sc[:], negp[:], None, OP.add)
                    u_t = sc_p.tile([P, E], F32, tag="u")
                    nc.vector.scalar_tensor_tensor(
                        out=u_t[:], in0=a_t[:], scalar=-1.0, in1=a_t[:],
                        op0=OP.mult, op1=OP.mult,
                    )
                    u8 = sm_p.tile([P, 8], F32, tag="u8")
                    idx8 = sm_p.tile([P, 8], U32, tag="idx8")
                    nc.vector.max(u8[:], u_t[:])
                    nc.vector.max_index(idx8[:], u8[:], u_t[:])
                    # gather codebook rows -> w_tilde
                    wt = wt_p.tile([P, A], F32, tag="wt")
                    nc.gpsimd.indirect_dma_start(
                        out=wt[:],
                        out_offset=None,
                        in_=cb_d[:, :],
                        in_offset=bass.IndirectOffsetOnAxis(ap=idx8[:, 0:1], axis=0),
                    )
                    # y_tilde = sum(x*w_tilde[:512]) + w_tilde[512]
                    scr2 = sm_p.tile([P, D], F32, tag="scr")
                    yp = sm_p.tile([P, 1], F32, tag="yp")
                    nc.vector.scalar_tensor_tensor(
                        out=scr2[:], in0=wt[:, 0:512], scalar=1.0, in1=xr[:],
                        op0=OP.mult, op1=OP.mult, accum_out=yp[:],
                    )
                    y_sb = sm_p.tile([P, 1], F32, tag="y")
                    nc.vector.tensor_tensor(y_sb[:], yp[:], wt[:, 512:513], op=OP.add)

                    nc.sync.dma_start(o_wh[row0:row0 + P, :], wh[:])
                    nc.sync.dma_start(o_wt[row0:row0 + P, :], wt[:])
                    nc.sync.dma_start(o_ix[row0:row0 + P, :], idx8[:, 0:1].bitcast(I32))
                    nc.sync.dma_start(o_y[row0:row0 + P, :], y_sb[:])

            # ---------------- main loop ----------------
            for t in range(NT):
                xt = xt_p.tile([P, KD, BT], F32, tag="xt")
                c0 = t * BT
                for k in range(KD):
                    nc.sync.dma_start(xt[:, k, :], xT_d[k * P:(k + 1) * P, c0:c0 + BT])
                h1t = mlp_layer(KD, xt, W1_d, b1_sb, h1_p, "h1")
                h2t = mlp_layer(KH, h1t, W2_d, b2_sb, h2_p, "h2")
                whs = l3(h2t)
                scores_epilogue(xt, whs, t)

    nc.finalize()
    return nc


_CACHE = {}


def _get_program():
    if "nc" not in _CACHE:
        _CACHE["nc"] = build_program()
    return _CACHE["nc"]


def _prep_host(inputs):
    x = np.ascontiguousarray(inputs["input_tensor"], dtype=np.float32)
    cb = np.ascontiguousarray(inputs["codebook"], dtype=np.float32)
    shared = {
        "W1": np.ascontiguousarray(inputs["W1"], dtype=np.float32),
        "W2": np.ascontiguousarray(inputs["W2"], dtype=np.float32),
        "W3": np.ascontiguousarray(inputs["W3"], dtype=np.float32),
        "b1": np.ascontiguousarray(inputs["b1"], dtype=np.float32),
        "b2": np.ascontiguousarray(inputs["b2"], dtype=np.float32),
        "b3": np.ascontiguousarray(inputs["b3"], dtype=np.float32),
        "cbT": np.ascontiguousarray(cb.T),
        "cb": cb,
    }
    in_maps = []
    for c in range(NCORES):
        xs = x[c * BSH:(c + 1) * BSH]
        m = dict(shared)
        m["x"] = np.ascontiguousarray(xs)
        m["xT"] = np.ascontiguousarray(xs.T)
        in_maps.append(m)
    return in_maps


def run(inputs, trace=False, **kwargs):
    nc = _get_program()
    in_maps = _prep_host(inputs)
    res = run_bass_kernel_spmd(
        nc, in_maps, core_ids=list(range(NCORES)), trace=trace, **kwargs
    )
    w_hat = np.concatenate([res.results[c]["w_hat"] for c in range(NCORES)], axis=0)
    w_tilde = np.concatenate([res.results[c]["w_tilde"] for c in range(NCORES)], axis=0)
    cluster = np.concatenate([res.results[c]["cluster"] for c in range(NCORES)], axis=0)
    y_tilde = np.concatenate([res.results[c]["y_tilde"] for c in range(NCORES)], axis=0)
    return (w_hat, w_tilde, cluster.astype(np.int32), y_tilde), res


def kernel(**inputs):
    outs, _ = run(inputs, trace=False)
    return outs


# revision 5
# speedup vs baseline: 1.0363x; 1.0363x over previous
"""Trainium2 Bass kernel for the vq_codebook problem (nn_CGCLR_72370198937695).

Math (per row b of the batch):
    aug    = [x_b, 1]                                   # [513]
    h1     = relu(x_b @ W1 + b1)                        # [2048]
    h2     = relu(h1 @ W2 + b2)                         # [2048]
    w_hat  = h2 @ W3 + b3                               # [513]
    proj   = aug . w_hat                                # scalar
    s_j    = aug . codebook_j                           # [1024]
    idx    = argmin_j (proj - s_j)^2  == argmax_j -(s_j - proj)^2
    w_tilde= codebook[idx]
    y_tilde= aug . w_tilde

Distribution: data-parallel over 8 NeuronCores, batch 16384 -> 2048 rows/core.
Weights + codebook replicated; no collectives. All matmuls fp32 (full
precision; fp32 runs at 4 cycles/row on the PE -> the kernel is PE-bound).

Per-core schedule, batch tiles of 512 (the matmul moving free dim):
    L1/L2 keep activations TRANSPOSED (feature-major):
        h1T[m-chunk] [128, 512] = (W1[k,m] chunk stationary).T @ xT-moving
    L3 + codebook scores are batch-major:
        w_hat rows = (h2T chunk stationary).T @ W3-moving
        scores rows = (xT chunk stationary).T @ cbT-moving
    argmin via DVE max/max_index on u = -(s - proj)^2
    w_tilde gathered from DRAM codebook by indirect DMA.
"""
import sys

for _p in ("/opt/trn_rl_repo", "/root/.axon_site"):
    if _p not in sys.path:
        sys.path.insert(0, _p)

import numpy as np
import concourse.bacc as bacc
import concourse.bass as bass
import concourse.tile as tile
from concourse import mybir
from concourse.bass_utils import run_bass_kernel_spmd

F32 = mybir.dt.float32
I32 = mybir.dt.int32
U32 = mybir.dt.uint32
AF = mybir.ActivationFunctionType
OP = mybir.AluOpType

NCORES = 8
B = 16384
D = 512       # input dim
H = 2048      # hidden dim
E = 1024      # codebook entries
A = 513       # aug dim = D + 1
P = 128       # partitions
BSH = B // NCORES   # rows per core = 2048
BT = 512            # batch tile (matmul moving free dim)
NT = BSH // BT      # 4 batch tiles per core
KD = D // P         # 4 contraction chunks for input dim
KH = H // P         # 16 contraction chunks for hidden dim
MH = H // P         # 16 output chunks per hidden layer
RG = BT // P        # 4 row-groups of 128 per batch tile


def _bcast(handle, offset, n_free):
    """DRAM AP broadcasting one row across all 128 partitions."""
    base = handle[:]
    return bass.AP(tensor=base.tensor, offset=offset, ap=[[0, P], [1, n_free]])


def build_program():
    nc = bacc.Bacc("TRN2", target_bir_lowering=False)

    x_d = nc.dram_tensor("x", [BSH, D], F32, kind="ExternalInput")
    xT_d = nc.dram_tensor("xT", [D, BSH], F32, kind="ExternalInput")
    W1_d = nc.dram_tensor("W1", [D, H], F32, kind="ExternalInput")
    W2_d = nc.dram_tensor("W2", [H, H], F32, kind="ExternalInput")
    W3_d = nc.dram_tensor("W3", [H, A], F32, kind="ExternalInput")
    b1_d = nc.dram_tensor("b1", [H], F32, kind="ExternalInput")
    b2_d = nc.dram_tensor("b2", [H], F32, kind="ExternalInput")
    b3_d = nc.dram_tensor("b3", [A], F32, kind="ExternalInput")
    cbT_d = nc.dram_tensor("cbT", [A, E], F32, kind="ExternalInput")
    cb_d = nc.dram_tensor("cb", [E, A], F32, kind="ExternalInput")

    o_wh = nc.dram_tensor("w_hat", [BSH, A], F32, kind="ExternalOutput")
    o_wt = nc.dram_tensor("w_tilde", [BSH, A], F32, kind="ExternalOutput")
    o_ix = nc.dram_tensor("cluster", [BSH, 1], I32, kind="ExternalOutput")
    o_y = nc.dram_tensor("y_tilde", [BSH, 1], F32, kind="ExternalOutput")

    with tile.TileContext(nc) as tc:
        with (
            tc.tile_pool(name="const", bufs=1) as const,
            tc.tile_pool(name="xt", bufs=2) as xt_p,
            tc.tile_pool(name="h1", bufs=1) as h1_p,
            tc.tile_pool(name="h2", bufs=1) as h2_p,
            tc.tile_pool(name="wp", bufs=3) as wp_p,
            tc.tile_pool(name="wh", bufs=6) as wh_p,
            tc.tile_pool(name="sc", bufs=2) as sc_p,
            tc.tile_pool(name="wt", bufs=2) as wt_p,
            tc.tile_pool(name="xr", bufs=2) as xr_p,
            tc.tile_pool(name="sm", bufs=3) as sm_p,
            tc.tile_pool(name="ps", bufs=8, space="PSUM") as ps_p,
        ):
            # ---------------- constants ----------------
            cbT_sb = const.tile([P, KD, E], F32)
            for k in range(KD):
                nc.sync.dma_start(cbT_sb[:, k, :], cbT_d[k * P:(k + 1) * P, :])
            c_rep = const.tile([P, E], F32)          # codebook[:,512] replicated
            nc.sync.dma_start(c_rep[:], _bcast(cbT_d, D * E, E))
            b3_rep = const.tile([P, A], F32)
            nc.sync.dma_start(b3_rep[:], _bcast(b3_d, 0, A))
            b1_sb = const.tile([P, KH], F32)         # b1_sb[p,m] = b1[m*128+p]
            nc.sync.dma_start(
                b1_sb[:],
                bass.AP(tensor=b1_d[:].tensor, offset=0, ap=[[1, P], [P, KH]]),
            )
            b2_sb = const.tile([P, KH], F32)
            nc.sync.dma_start(
                b2_sb[:],
                bass.AP(tensor=b2_d[:].tensor, offset=0, ap=[[1, P], [P, KH]]),
            )
            w3l_sb = const.tile([P, KH], F32)    # w3l[p,k] = W3[k*128+p, 512]
            nc.sync.dma_start(
                w3l_sb[:],
                bass.AP(tensor=W3_d[:].tensor, offset=D, ap=[[A, P], [A * P, KH]]),
            )
            ones_sb = const.tile([P, 1], F32)
            nc.vector.memset(ones_sb[:], 1.0)

            # ---------------- per-tile pipeline ----------------
            def mlp_layer(KC, rhs_tile, W_dram, bias_sb, out_pool, out_tag):
                """out (feature-major [128, MH, BT] f32) = relu(W-chunk.T @ rhs + b)."""
                out = out_pool.tile([P, MH, BT], F32, tag=out_tag)
                for mg in range(4):  # groups of 4 m-chunks -> 4 live psum banks
                    pss = [ps_p.tile([P, BT], F32, tag="mm", name=f"ps_l{KC}_{mg}_{i}") for i in range(4)]
                    for k in range(KC):
                        wp = wp_p.tile([P, 512], F32, tag="wp")
                        nc.sync.dma_start(
                            wp[:],
                            W_dram[k * P:(k + 1) * P, mg * 512:(mg + 1) * 512],
                        )
                        for m in range(4):
                            nc.tensor.matmul(
                                pss[m][:],
                                wp[:, m * P:(m + 1) * P],
                                rhs_tile[:, k, :],
                                start=(k == 0),
                                stop=(k == KC - 1),
                            )
                    for m in range(4):
                        mi = mg * 4 + m
                        nc.scalar.activation(
                            out[:, mi, :],
                            pss[m][:],
                            AF.Relu,
                            bias=bias_sb[:, mi:mi + 1],
                        )
                return out

            def l3(h2t):
                """w_hat batch-major per rg: [128, 513] tiles."""
                whs = [wh_p.tile([P, A], F32, tag="wh", name=f"wh_{i}") for i in range(RG)]
                pss = [ps_p.tile([P, BT], F32, tag="mm", name=f"ps_l3_{i}") for i in range(RG)]
                for k in range(KH):
                    wp = wp_p.tile([P, A], F32, tag="wp")
                    nc.sync.dma_start(wp[:], W3_d[k * P:(k + 1) * P, :])
                    for rg in range(RG):
                        nc.tensor.matmul(
                            pss[rg][:],
                            h2t[:, k, rg * P:(rg + 1) * P],
                            wp[:, 0:512],
                            start=(k == 0),
                            stop=(k == KH - 1),
                        )
                for rg in range(RG):
                    nc.vector.tensor_tensor(
                        whs[rg][:, 0:512], pss[rg][:], b3_rep[:, 0:512], op=OP.add
                    )
                # tail column 512: DVE accumulate over k-chunks, then a single
                # ones-vector matmul reduces the partition dim -> [1, BT]
                acc = sm_p.tile([P, BT], F32, tag="tacc")
                nc.vector.tensor_scalar(acc[:], h2t[:, 0, :], w3l_sb[:, 0:1], None, OP.mult)
                for k in range(1, KH):
                    nc.vector.scalar_tensor_tensor(
                        out=acc[:], in0=h2t[:, k, :], scalar=w3l_sb[:, k:k + 1],
                        in1=acc[:], op0=OP.mult, op1=OP.add,
                    )
                pst = ps_p.tile([1, BT], F32, tag="mm", name="ps_tail")
                nc.tensor.matmul(pst[:], ones_sb[:], acc[:], start=True, stop=True)
                tailT = sm_p.tile([1, BT], F32, tag="tailT")
                nc.vector.tensor_scalar(
                    tailT[:], pst[:], b3_rep[0:1, 512:513], None, OP.add
                )
                for rg in range(RG):
                    nc.sync.dma_start(
                        whs[rg][:, 512:513], tailT[0:1, rg * P:(rg + 1) * P]
                    )
                return whs

            def scores_epilogue(xt, whs, t):
                for rg in range(RG):
                    row0 = t * BT + rg * P
                    wh = whs[rg]
                    sc = sc_p.tile([P, E], F32, tag="sc")
                    for nb in range(2):
                        ps = ps_p.tile([P, BT], F32, tag="mm")
                        for k in range(KD):
                            nc.tensor.matmul(
                                ps[:],
                                xt[:, k, rg * P:(rg + 1) * P],
                                cbT_sb[:, k, nb * 512:(nb + 1) * 512],
                                start=(k == 0),
                                stop=(k == KD - 1),
                            )
                        nc.vector.tensor_tensor(
                            sc[:, nb * 512:(nb + 1) * 512],
                            ps[:],
                            c_rep[:, nb * 512:(nb + 1) * 512],
                            op=OP.add,
                        )
                    xr = xr_p.tile([P, D], F32, tag="xr")
                    nc.sync.dma_start(xr[:], x_d[row0:row0 + P, :])
                    # proj = sum(x*w_hat[:512]) + w_hat[512]
                    scr = sm_p.tile([P, D], F32, tag="scr")
                    projp = sm_p.tile([P, 1], F32, tag="projp")
                    nc.vector.scalar_tensor_tensor(
                        out=scr[:], in0=wh[:, 0:512], scalar=1.0, in1=xr[:],
                        op0=OP.mult, op1=OP.mult, accum_out=projp[:],
                    )
                    negp = sm_p.tile([P, 1], F32, tag="negp")
                    nc.vector.tensor_scalar(
                        negp[:], projp[:], wh[:, 512:513], -1.0, OP.add, OP.mult
                    )
                    # u = -(s - proj)^2 ; argmax u == argmin distance
                    a_t = sc_p.tile([P, E], F32, tag="a")
                    nc.gpsimd.tensor_scalar(a_t[:], sc[:], negp[:], None, OP.add)
                    u_t = sc_p.tile([P, E], F32, tag="u")
                    nc.vector.scalar_tensor_tensor(
                        out=u_t[:], in0=a_t[:], scalar=-1.0, in1=a_t[:],
                        op0=OP.mult, op1=OP.mult,
                    )
                    u8 = sm_p.tile([P, 8], F32, tag="u8")
                    idx8 = sm_p.tile([P, 8], U32, tag="idx8")
                    nc.vector.max(u8[:], u_t[:])
                    nc.vector.max_index(idx8[:], u8[:], u_t[:])
                    # gather codebook rows -> w_tilde
                    wt = wt_p.tile([P, A], F32, tag="wt")
                    nc.gpsimd.indirect_dma_start(
                        out=wt[:],
                        out_offset=None,
                        in_=cb_d[:, :],
                        in_offset=bass.IndirectOffsetOnAxis(ap=idx8[:, 0:1], axis=0),
                    )
                    # y_tilde = sum(x*w_tilde[:512]) + w_tilde[512]
                    scr2 = sm_p.tile([P, D], F32, tag="scr")
                    yp = sm_p.tile([P, 1], F32, tag="yp")
                    nc.vector.scalar_tensor_tensor(
                        out=scr2[:], in0=wt[:, 0:512], scalar=1.0, in1=xr[:],
                        op0=OP.mult, op1=OP.mult, accum_out=yp[:],
                    )
                    y_sb = sm_p.tile([P, 1], F32, tag="y")
                    nc.vector.tensor_tensor(y_sb[:], yp[:], wt[:, 512:513], op=OP.add)

                    nc.sync.dma_start(o_wh[row0:row0 + P, :], wh[:])
                    nc.sync.dma_start(o_wt[row0:row0 + P, :], wt[:])
                    nc.sync.dma_start(o_ix[row0:row0 + P, :], idx8[:, 0:1].bitcast(I32))
                    nc.sync.dma_start(o_y[row0:row0 + P, :], y_sb[:])

            # ---------------- main loop ----------------
            for t in range(NT):
                xt = xt_p.tile([P, KD, BT], F32, tag="xt")
                c0 = t * BT
                for k in range(KD):
                    nc.sync.dma_start(xt[:, k, :], xT_d[k * P:(k + 1) * P, c0:c0 + BT])
                h1t = mlp_layer(KD, xt, W1_d, b1_sb, h1_p, "h1")
                h2t = mlp_layer(KH, h1t, W2_d, b2_sb, h2_p, "h2")
                whs = l3(h2t)
                scores_epilogue(xt, whs, t)

    nc.finalize()
    return nc


_CACHE = {}


def _get_program():
    if "nc" not in _CACHE:
        _CACHE["nc"] = build_program()
    return _CACHE["nc"]


def _prep_host(inputs):
    x = np.ascontiguousarray(inputs["input_tensor"], dtype=np.float32)
    cb = np.ascontiguousarray(inputs["codebook"], dtype=np.float32)
    shared = {
        "W1": np.ascontiguousarray(inputs["W1"], dtype=np.float32),
        "W2": np.ascontiguousarray(inputs["W2"], dtype=np.float32),
        "W3": np.ascontiguousarray(inputs["W3"], dtype=np.float32),
        "b1": np.ascontiguousarray(inputs["b1"], dtype=np.float32),
        "b2": np.ascontiguousarray(inputs["b2"], dtype=np.float32),
        "b3": np.ascontiguousarray(inputs["b3"], dtype=np.float32),
        "cbT": np.ascontiguousarray(cb.T),
        "cb": cb,
    }
    in_maps = []
    for c in range(NCORES):
        xs = x[c * BSH:(c + 1) * BSH]
        m = dict(shared)
        m["x"] = np.ascontiguousarray(xs)
        m["xT"] = np.ascontiguousarray(xs.T)
        in_maps.append(m)
    return in_maps


def run(inputs, trace=False, **kwargs):
    nc = _get_program()
    in_maps = _prep_host(inputs)
    res = run_bass_kernel_spmd(
        nc, in_maps, core_ids=list(range(NCORES)), trace=trace, **kwargs
    )
    w_hat = np.concatenate([res.results[c]["w_hat"] for c in range(NCORES)], axis=0)
    w_tilde = np.concatenate([res.results[c]["w_tilde"] for c in range(NCORES)], axis=0)
    cluster = np.concatenate([res.results[c]["cluster"] for c in range(NCORES)], axis=0)
    y_tilde = np.concatenate([res.results[c]["y_tilde"] for c in range(NCORES)], axis=0)
    return (w_hat, w_tilde, cluster.astype(np.int32), y_tilde), res


def kernel(**inputs):
    outs, _ = run(inputs, trace=False)
    return outs


# revision 7
# speedup vs baseline: 1.1561x; 1.1156x over previous
"""Trainium2 Bass kernel for the vq_codebook problem (nn_CGCLR_72370198937695).

Math (per row b of the batch):
    aug    = [x_b, 1]                                   # [513]
    h1     = relu(x_b @ W1 + b1)                        # [2048]
    h2     = relu(h1 @ W2 + b2)                         # [2048]
    w_hat  = h2 @ W3 + b3                               # [513]
    proj   = aug . w_hat                                # scalar
    s_j    = aug . codebook_j                           # [1024]
    idx    = argmin_j (proj - s_j)^2  == argmax_j -(s_j - proj)^2
    w_tilde= codebook[idx]
    y_tilde= aug . w_tilde

Distribution: data-parallel over 8 NeuronCores, batch 16384 -> 2048 rows/core.
Weights + codebook replicated; no collectives. All matmuls fp32 (full
precision; fp32 runs at 4 cycles/row on the PE -> the kernel is PE-bound).

Per-core schedule, batch tiles of 512 (the matmul moving free dim):
    L1/L2 keep activations TRANSPOSED (feature-major):
        h1T[m-chunk] [128, 512] = (W1[k,m] chunk stationary).T @ xT-moving
    L3 + codebook scores are batch-major:
        w_hat rows = (h2T chunk stationary).T @ W3-moving
        scores rows = (xT chunk stationary).T @ cbT-moving
    argmin via DVE max/max_index on u = -(s - proj)^2
    w_tilde gathered from DRAM codebook by indirect DMA.
"""
import sys

for _p in ("/opt/trn_rl_repo", "/root/.axon_site"):
    if _p not in sys.path:
        sys.path.insert(0, _p)

import numpy as np
import concourse.bacc as bacc
import concourse.bass as bass
import concourse.tile as tile
from concourse import mybir
from concourse.bass_utils import run_bass_kernel_spmd

F32 = mybir.dt.float32
I32 = mybir.dt.int32
U32 = mybir.dt.uint32
AF = mybir.ActivationFunctionType
OP = mybir.AluOpType

NCORES = 8
B = 16384
D = 512       # input dim
H = 2048      # hidden dim
E = 1024      # codebook entries
A = 513       # aug dim = D + 1
P = 128       # partitions
BSH = B // NCORES   # rows per core = 2048
BT = 512            # batch tile (matmul moving free dim)
NT = BSH // BT      # 4 batch tiles per core
KD = D // P         # 4 contraction chunks for input dim
KH = H // P         # 16 contraction chunks for hidden dim
MH = H // P         # 16 output chunks per hidden layer
RG = BT // P        # 4 row-groups of 128 per batch tile


def _bcast(handle, offset, n_free):
    """DRAM AP broadcasting one row across all 128 partitions."""
    base = handle[:]
    return bass.AP(tensor=base.tensor, offset=offset, ap=[[0, P], [1, n_free]])


def build_program():
    nc = bacc.Bacc("TRN2", target_bir_lowering=False)

    x_d = nc.dram_tensor("x", [BSH, D], F32, kind="ExternalInput")
    xT_d = nc.dram_tensor("xT", [D, BSH], F32, kind="ExternalInput")
    W1_d = nc.dram_tensor("W1", [D, H], F32, kind="ExternalInput")
    W2_d = nc.dram_tensor("W2", [H, H], F32, kind="ExternalInput")
    W3_d = nc.dram_tensor("W3", [H, A], F32, kind="ExternalInput")
    b1_d = nc.dram_tensor("b1", [H], F32, kind="ExternalInput")
    b2_d = nc.dram_tensor("b2", [H], F32, kind="ExternalInput")
    b3_d = nc.dram_tensor("b3", [A], F32, kind="ExternalInput")
    cbT_d = nc.dram_tensor("cbT", [A, E], F32, kind="ExternalInput")
    cb_d = nc.dram_tensor("cb", [E, A], F32, kind="ExternalInput")

    o_wh = nc.dram_tensor("w_hat", [BSH, A], F32, kind="ExternalOutput")
    o_wt = nc.dram_tensor("w_tilde", [BSH, A], F32, kind="ExternalOutput")
    o_ix = nc.dram_tensor("cluster", [BSH, 1], I32, kind="ExternalOutput")
    o_y = nc.dram_tensor("y_tilde", [BSH, 1], F32, kind="ExternalOutput")

    with tile.TileContext(nc) as tc:
        with (
            tc.tile_pool(name="const", bufs=1) as const,
            tc.tile_pool(name="xt", bufs=2) as xt_p,
            tc.tile_pool(name="h1", bufs=1) as h1_p,
            tc.tile_pool(name="h2", bufs=1) as h2_p,
            tc.tile_pool(name="wp", bufs=3) as wp_p,
            tc.tile_pool(name="wh", bufs=6) as wh_p,
            tc.tile_pool(name="sc", bufs=2) as sc_p,
            tc.tile_pool(name="wt", bufs=2) as wt_p,
            tc.tile_pool(name="xr", bufs=2) as xr_p,
            tc.tile_pool(name="sm", bufs=3) as sm_p,
            tc.tile_pool(name="ps", bufs=8, space="PSUM") as ps_p,
        ):
            # ---------------- constants ----------------
            cbT_sb = const.tile([P, KD, E], F32)
            for k in range(KD):
                nc.sync.dma_start(cbT_sb[:, k, :], cbT_d[k * P:(k + 1) * P, :])
            c_rep = const.tile([P, E], F32)          # codebook[:,512] replicated
            nc.sync.dma_start(c_rep[:], _bcast(cbT_d, D * E, E))
            b3_rep = const.tile([P, A], F32)
            nc.sync.dma_start(b3_rep[:], _bcast(b3_d, 0, A))
            b1_sb = const.tile([P, KH], F32)         # b1_sb[p,m] = b1[m*128+p]
            nc.sync.dma_start(
                b1_sb[:],
                bass.AP(tensor=b1_d[:].tensor, offset=0, ap=[[1, P], [P, KH]]),
            )
            b2_sb = const.tile([P, KH], F32)
            nc.sync.dma_start(
                b2_sb[:],
                bass.AP(tensor=b2_d[:].tensor, offset=0, ap=[[1, P], [P, KH]]),
            )
            w3l_sb = const.tile([P, KH], F32)    # w3l[p,k] = W3[k*128+p, 512]
            nc.sync.dma_start(
                w3l_sb[:],
                bass.AP(tensor=W3_d[:].tensor, offset=D, ap=[[A, P], [A * P, KH]]),
            )
            ones_sb = const.tile([P, 1], F32)
            nc.vector.memset(ones_sb[:], 1.0)

            # ---------------- per-tile pipeline ----------------
            def mlp_layer(KC, rhs_tile, W_dram, bias_sb, out_pool, out_tag):
                """out (feature-major [128, MH, BT] f32) = relu(W-chunk.T @ rhs + b)."""
                out = out_pool.tile([P, MH, BT], F32, tag=out_tag)
                for mg in range(4):  # groups of 4 m-chunks -> 4 live psum banks
                    pss = [ps_p.tile([P, BT], F32, tag="mm", name=f"ps_l{KC}_{mg}_{i}") for i in range(4)]
                    for k in range(KC):
                        wp = wp_p.tile([P, 512], F32, tag="wp")
                        nc.sync.dma_start(
                            wp[:],
                            W_dram[k * P:(k + 1) * P, mg * 512:(mg + 1) * 512],
                        )
                        for m in range(4):
                            nc.tensor.matmul(
                                pss[m][:],
                                wp[:, m * P:(m + 1) * P],
                                rhs_tile[:, k, :],
                                start=(k == 0),
                                stop=(k == KC - 1),
                            )
                    for m in range(4):
                        mi = mg * 4 + m
                        nc.scalar.activation(
                            out[:, mi, :],
                            pss[m][:],
                            AF.Relu,
                            bias=bias_sb[:, mi:mi + 1],
                        )
                return out

            def l3(h2t):
                """w_hat batch-major per rg: [128, 513] tiles."""
                whs = [wh_p.tile([P, A], F32, tag="wh", name=f"wh_{i}") for i in range(RG)]
                pss = [ps_p.tile([P, BT], F32, tag="mm", name=f"ps_l3_{i}") for i in range(RG)]
                for k in range(KH):
                    wp = wp_p.tile([P, A], F32, tag="wp")
                    nc.sync.dma_start(wp[:], W3_d[k * P:(k + 1) * P, :])
                    for rg in range(RG):
                        nc.tensor.matmul(
                            pss[rg][:],
                            h2t[:, k, rg * P:(rg + 1) * P],
                            wp[:, 0:512],
                            start=(k == 0),
                            stop=(k == KH - 1),
                        )
                for rg in range(RG):
                    nc.vector.tensor_tensor(
                        whs[rg][:, 0:512], pss[rg][:], b3_rep[:, 0:512], op=OP.add
                    )
                # tail column 512: DVE accumulate over k-chunks, then a single
                # ones-vector matmul reduces the partition dim -> [1, BT]
                acc = sm_p.tile([P, BT], F32, tag="tacc")
                nc.vector.tensor_scalar(acc[:], h2t[:, 0, :], w3l_sb[:, 0:1], None, OP.mult)
                for k in range(1, KH):
                    nc.vector.scalar_tensor_tensor(
                        out=acc[:], in0=h2t[:, k, :], scalar=w3l_sb[:, k:k + 1],
                        in1=acc[:], op0=OP.mult, op1=OP.add,
                    )
                pst = ps_p.tile([1, BT], F32, tag="mm", name="ps_tail")
                nc.tensor.matmul(pst[:], ones_sb[:], acc[:], start=True, stop=True)
                tailT = sm_p.tile([1, BT], F32, tag="tailT")
                nc.vector.tensor_scalar(
                    tailT[:], pst[:], b3_rep[0:1, 512:513], None, OP.add
                )
                for rg in range(RG):
                    nc.sync.dma_start(
                        whs[rg][:, 512:513], tailT[0:1, rg * P:(rg + 1) * P]
                    )
                return whs

            def scores_epilogue(xt, whs, t):
                # pass 1: score matmuls + psum-freeing evacuations first, so
                # the next tile's matmuls never wait on the argmin chains
                scs, xrs = [], []
                for rg in range(RG):
                    row0 = t * BT + rg * P
                    sc = sc_p.tile([P, E], F32, tag="sc", name=f"sc_{rg}", bufs=5)
                    scs.append(sc)
                    for nb in range(2):
                        ps = ps_p.tile([P, BT], F32, tag="mm", name=f"ps_sc_{rg}_{nb}")
                        for k in range(KD):
                            nc.tensor.matmul(
                                ps[:],
                                xt[:, k, rg * P:(rg + 1) * P],
                                cbT_sb[:, k, nb * 512:(nb + 1) * 512],
                                start=(k == 0),
                                stop=(k == KD - 1),
                            )
                        nc.vector.tensor_tensor(
                            sc[:, nb * 512:(nb + 1) * 512],
                            ps[:],
                            c_rep[:, nb * 512:(nb + 1) * 512],
                            op=OP.add,
                        )
                    xr = xr_p.tile([P, D], F32, tag="xr", name=f"xr_{rg}", bufs=5)
                    nc.sync.dma_start(xr[:], x_d[row0:row0 + P, :])
                    xrs.append(xr)
                # pass 2: per-rg argmin/gather/output chains
                for rg in range(RG):
                    row0 = t * BT + rg * P
                    wh = whs[rg]
                    sc = scs[rg]
                    xr = xrs[rg]
                    # proj = sum(x*w_hat[:512]) + w_hat[512]
                    scr = sm_p.tile([P, D], F32, tag="scr")
                    projp = sm_p.tile([P, 1], F32, tag="projp")
                    nc.vector.scalar_tensor_tensor(
                        out=scr[:], in0=wh[:, 0:512], scalar=1.0, in1=xr[:],
                        op0=OP.mult, op1=OP.mult, accum_out=projp[:],
                    )
                    negp = sm_p.tile([P, 1], F32, tag="negp")
                    nc.vector.tensor_scalar(
                        negp[:], projp[:], wh[:, 512:513], -1.0, OP.add, OP.mult
                    )
                    # u = -(s - proj)^2 ; argmax u == argmin distance
                    a_t = sc_p.tile([P, E], F32, tag="a")
                    nc.vector.tensor_scalar(a_t[:], sc[:], negp[:], None, OP.add)
                    u_t = sc_p.tile([P, E], F32, tag="u")
                    nc.vector.scalar_tensor_tensor(
                        out=u_t[:], in0=a_t[:], scalar=-1.0, in1=a_t[:],
                        op0=OP.mult, op1=OP.mult,
                    )
                    u8 = sm_p.tile([P, 8], F32, tag="u8")
                    idx8 = sm_p.tile([P, 8], U32, tag="idx8")
                    nc.vector.max(u8[:], u_t[:])
                    nc.vector.max_index(idx8[:], u8[:], u_t[:])
                    # gather codebook rows -> w_tilde
                    wt = wt_p.tile([P, A], F32, tag="wt")
                    nc.gpsimd.indirect_dma_start(
                        out=wt[:],
                        out_offset=None,
                        in_=cb_d[:, :],
                        in_offset=bass.IndirectOffsetOnAxis(ap=idx8[:, 0:1], axis=0),
                    )
                    # y_tilde = sum(x*w_tilde[:512]) + w_tilde[512]
                    scr2 = sm_p.tile([P, D], F32, tag="scr")
                    yp = sm_p.tile([P, 1], F32, tag="yp")
                    nc.vector.scalar_tensor_tensor(
                        out=scr2[:], in0=wt[:, 0:512], scalar=1.0, in1=xr[:],
                        op0=OP.mult, op1=OP.mult, accum_out=yp[:],
                    )
                    y_sb = sm_p.tile([P, 1], F32, tag="y")
                    nc.vector.tensor_tensor(y_sb[:], yp[:], wt[:, 512:513], op=OP.add)

                    nc.sync.dma_start(o_wh[row0:row0 + P, :], wh[:])
                    nc.sync.dma_start(o_wt[row0:row0 + P, :], wt[:])
                    nc.sync.dma_start(o_ix[row0:row0 + P, :], idx8[:, 0:1].bitcast(I32))
                    nc.sync.dma_start(o_y[row0:row0 + P, :], y_sb[:])

            # ---------------- main loop ----------------
            for t in range(NT):
                xt = xt_p.tile([P, KD, BT], F32, tag="xt")
                c0 = t * BT
                for k in range(KD):
                    nc.sync.dma_start(xt[:, k, :], xT_d[k * P:(k + 1) * P, c0:c0 + BT])
                h1t = mlp_layer(KD, xt, W1_d, b1_sb, h1_p, "h1")
                h2t = mlp_layer(KH, h1t, W2_d, b2_sb, h2_p, "h2")
                whs = l3(h2t)
                scores_epilogue(xt, whs, t)

    nc.finalize()
    return nc


_CACHE = {}


def _get_program():
    if "nc" not in _CACHE:
        _CACHE["nc"] = build_program()
    return _CACHE["nc"]


def _prep_host(inputs):
    x = np.ascontiguousarray(inputs["input_tensor"], dtype=np.float32)
    cb = np.ascontiguousarray(inputs["codebook"], dtype=np.float32)
    shared = {
        "W1": np.ascontiguousarray(inputs["W1"], dtype=np.float32),
        "W2": np.ascontiguousarray(inputs["W2"], dtype=np.float32),
        "W3": np.ascontiguousarray(inputs["W3"], dtype=np.float32),
        "b1": np.ascontiguousarray(inputs["b1"], dtype=np.float32),
        "b2": np.ascontiguousarray(inputs["b2"], dtype=np.float32),
        "b3": np.ascontiguousarray(inputs["b3"], dtype=np.float32),
        "cbT": np.ascontiguousarray(cb.T),
        "cb": cb,
    }
    in_maps = []
    for c in range(NCORES):
        xs = x[c * BSH:(c + 1) * BSH]
        m = dict(shared)
        m["x"] = np.ascontiguousarray(xs)
        m["xT"] = np.ascontiguousarray(xs.T)
        in_maps.append(m)
    return in_maps


def run(inputs, trace=False, **kwargs):
    nc = _get_program()
    in_maps = _prep_host(inputs)
    res = run_bass_kernel_spmd(
        nc, in_maps, core_ids=list(range(NCORES)), trace=trace, **kwargs
    )
    w_hat = np.concatenate([res.results[c]["w_hat"] for c in range(NCORES)], axis=0)
    w_tilde = np.concatenate([res.results[c]["w_tilde"] for c in range(NCORES)], axis=0)
    cluster = np.concatenate([res.results[c]["cluster"] for c in range(NCORES)], axis=0)
    y_tilde = np.concatenate([res.results[c]["y_tilde"] for c in range(NCORES)], axis=0)
    return (w_hat, w_tilde, cluster.astype(np.int32), y_tilde), res


def kernel(**inputs):
    outs, _ = run(inputs, trace=False)
    return outs
